# revision 15
# baseline (speedup 1.0000x reference)
"""Expert-parallel MoE (Kimi/DeepSeek-V3 style sparse block) on 8 trn2 NeuronCores.

Strategy (v5 — balanced, bf16, DMA-lean):
  - Host computes the sigmoid gate + group-limited top-2 routing in numpy
    float64, then packs a balanced per-core schedule of four 512-token
    matmul chunks: [own-expert 512]x2  [shared 512]x2.
    Core c owns expert c's first 1024 tokens; the small per-expert overflow
    beyond 1024 (~2% of token-expert pairs for a balanced router) is
    evaluated on the host.  The shared FFN (SH=1024) is split into two F=512
    halves; cores 0-3 run half 0 and cores 4-7 run half 1, each over a
    distinct 1024-token slice, summed on the host.
  - All weights/activations ship as bf16 in the exact SBUF layout; matmuls
    accumulate in fp32 PSUM; outputs return bf16.
  - Overlap tuning: chunk 0's weight/x loads are split per contraction tile
    and its loops run k-outer so the PE streams at DMA arrival rate; warm-up
    matmuls on a memset tile bridge the preamble (HAM stays armed); output
    DMAs are batched 2-per-chunk (4 for the last chunk) to keep the Sync
    issue queue and the drain tail short.
"""

from contextlib import ExitStack

import numpy as np
import ml_dtypes

import concourse.bacc as bacc
import concourse.tile as tile
import concourse.mybir as mybir
from concourse import bass_utils

# --- model dims (hardcoded per problem spec) ---
B, S, D = 2, 2048, 1024
T = B * S                 # 4096 tokens
E, F = 8, 512             # routed experts / expert intermediate
SH = 1024                 # shared intermediate
TOP_K, N_GROUP, TOPK_GROUP = 2, 4, 2
SCALE = 2.5

N_CORES = 8
P = 128                   # SBUF partitions
NT = 512                  # token chunk (matmul moving free dim)
KD = D // P               # 8 contraction tiles for D
KF = F // P               # 4 contraction tiles for F
ACAP = 2 * NT             # own-expert capacity per core (2 chunks)
SSLICE = 2 * NT           # shared tokens per core (2 chunks)

F32 = mybir.dt.float32
BF16 = mybir.dt.bfloat16
NPBF16 = ml_dtypes.bfloat16

_CACHE: dict = {}


def _emit(nc):
    """Per-core program: 2 own-expert chunks + 2 shared chunks."""
    wa13 = nc.dram_tensor("wa13", [KD, P, 2 * F], BF16, kind="ExternalInput").ap()
    wa2 = nc.dram_tensor("wa2", [2, P, 2, D], BF16, kind="ExternalInput").ap()
    ws13 = nc.dram_tensor("ws13", [P, KD, 2 * F], BF16, kind="ExternalInput").ap()
    ws2 = nc.dram_tensor("ws2", [P, KF, D], BF16, kind="ExternalInput").ap()
    xa0 = nc.dram_tensor("xa0", [KD, P, NT], BF16, kind="ExternalInput").ap()
    xa1 = nc.dram_tensor("xa1", [P, KD, NT], BF16, kind="ExternalInput").ap()
    xs = nc.dram_tensor("xs", [2, P, KD, NT], BF16, kind="ExternalInput").ap()
    ya = nc.dram_tensor("ya", [2, P, KD, NT], BF16, kind="ExternalOutput").ap()
    ys = nc.dram_tensor("ys", [2, P, KD, NT], BF16, kind="ExternalOutput").ap()

    silu = mybir.ActivationFunctionType.Silu

    with tile.TileContext(nc) as tc, ExitStack() as ctx:
        pool = ctx.enter_context(tc.tile_pool(name="sb", bufs=1))
        pspool = ctx.enter_context(tc.tile_pool(name="ps", bufs=1, space="PSUM"))

        ps_tags = [f"ps{i}" for i in range(8)]

        # --- HAM warm-up: matmuls on a zero tile bridge the DMA head ---
        warm = pool.tile([P, NT], BF16, name="warm", tag="warm")
        nc.any.memset(warm[:], 0)
        wps = pspool.tile([P, NT], F32, name="wps", tag=ps_tags[0])
        for _ in range(6):
            nc.tensor.matmul(wps[:], warm[:, 0:P], warm[:], start=True, stop=True)

        # --- loads: chunk 0 split per k-tile so the PE streams immediately ---
        # (k-major DRAM layouts keep every piece transfer fully contiguous)
        w13a_p, xa0_p = [], []
        wa2lo = pool.tile([P, 2, D], BF16, name="wa2lo", tag="wa2lo")
        wa2hi = pool.tile([P, 2, D], BF16, name="wa2hi", tag="wa2hi")
        w1p0 = pool.tile([P, F], BF16, name="w1p0", tag="w1p0")
        nc.sync.dma_start(w1p0[:], wa13[0, :, 0:F])
        t = pool.tile([P, NT], BF16, name="xa0p0", tag="xa0p0")
        nc.sync.dma_start(t[:], xa0[0])
        xa0_p.append(t)
        w3p0 = pool.tile([P, F], BF16, name="w3p0", tag="w3p0")
        nc.sync.dma_start(w3p0[:], wa13[0, :, F:2 * F])
        w13a_p.append(None)
        for k in range(1, KD):
            t = pool.tile([P, 2 * F], BF16, name=f"wa13p{k}", tag=f"wa13p{k}")
            nc.sync.dma_start(t[:], wa13[k])
            w13a_p.append(t)
            t = pool.tile([P, NT], BF16, name=f"xa0p{k}", tag=f"xa0p{k}")
            nc.sync.dma_start(t[:], xa0[k])
            xa0_p.append(t)
            if k == 5:
                nc.sync.dma_start(wa2lo[:], wa2[0])
        nc.sync.dma_start(wa2hi[:], wa2[1])
        xa1sb = pool.tile([P, KD, NT], BF16, name="xa1sb", tag="xa1")
        nc.sync.dma_start(xa1sb[:], xa1)
        ws13sb = pool.tile([P, KD, 2 * F], BF16, name="ws13sb", tag="ws13")
        nc.sync.dma_start(ws13sb[:], ws13)
        xs0sb = pool.tile([P, KD, NT], BF16, name="xs0sb", tag="xs0")
        nc.sync.dma_start(xs0sb[:], xs[0])
        ws2sb = pool.tile([P, KF, D], BF16, name="ws2sb", tag="ws2")
        nc.sync.dma_start(ws2sb[:], ws2)
        xs1sb = pool.tile([P, KD, NT], BF16, name="xs1sb", tag="xs1")
        nc.sync.dma_start(xs1sb[:], xs[1])

        def up_mfk(w1_at, w3_at, x_at, n):
            """Up-projection, mf-outer (pipelines silu/mul into the stream)."""
            hts = []
            for mf in range(KF):
                h1 = pspool.tile([P, n], F32, name="h1ps", tag=ps_tags[2 * mf])
                for k in range(KD):
                    nc.tensor.matmul(
                        h1[:], w1_at(k)[:, mf * P:(mf + 1) * P], x_at(k),
                        start=(k == 0), stop=(k == KD - 1),
                    )
                h3 = pspool.tile([P, n], F32, name="h3ps", tag=ps_tags[2 * mf + 1])
                for k in range(KD):
                    nc.tensor.matmul(
                        h3[:], w3_at(k)[:, mf * P:(mf + 1) * P], x_at(k),
                        start=(k == 0), stop=(k == KD - 1),
                    )
                a = pool.tile([P, n], F32, name="asb", tag=f"a{mf % 2}")
                nc.scalar.activation(a[:], h1[:], silu)
                ht = pool.tile([P, n], BF16, name="htsb", tag=f"ht{mf}")
                nc.vector.tensor_mul(ht[:], a[:], h3[:])
                hts.append(ht)
            return hts

        def up_kmf(w1_at, w3_at, x_at, n):
            """Up-projection, k-outer (streams at DMA arrival rate)."""
            h1s, h3s = [], []
            for k in range(KD):
                for mf in range(KF):
                    if k == 0:
                        h1s.append(pspool.tile([P, n], F32, name="h1ps",
                                               tag=ps_tags[mf]))
                    nc.tensor.matmul(
                        h1s[mf][:], w1_at(k)[:, mf * P:(mf + 1) * P], x_at(k),
                        start=(k == 0), stop=(k == KD - 1),
                    )
                for mf in range(KF):
                    if k == 0:
                        h3s.append(pspool.tile([P, n], F32, name="h3ps",
                                               tag=ps_tags[4 + mf]))
                    nc.tensor.matmul(
                        h3s[mf][:], w3_at(k)[:, mf * P:(mf + 1) * P],
                        x_at(k),
                        start=(k == 0), stop=(k == KD - 1),
                    )
            hts = []
            for mf in range(KF):
                a = pool.tile([P, n], F32, name="asb", tag=f"a{mf % 2}")
                nc.scalar.activation(a[:], h1s[mf][:], silu)
                ht = pool.tile([P, n], BF16, name="htsb", tag=f"ht{mf}")
                nc.vector.tensor_mul(ht[:], a[:], h3s[mf][:])
                hts.append(ht)
            return hts

        def down_md(w2_at, hts, n, ci, y_ap, batches):
            """Down-projection, md-outer; output DMAs batched per `batches`."""
            osb = pool.tile([P, KD, n], BF16, name="osb", tag=f"osb{ci % 2}")
            for md in range(KD):
                yps = pspool.tile([P, n], F32, name="yps", tag=ps_tags[md])
                for kf in range(KF):
                    nc.tensor.matmul(
                        yps[:], w2_at(kf)[:, md * P:(md + 1) * P], hts[kf][:],
                        start=(kf == 0), stop=(kf == KF - 1),
                    )
                nc.vector.tensor_copy(osb[:, md, :], yps[:])
                if md + 1 in batches:
                    lo = batches[md + 1]
                    nc.sync.dma_start(y_ap[:, lo:md + 1, :], osb[:, lo:md + 1, :])

        def down_kf(w2_at, hts, n, ci, y_ap, batches):
            """Down-projection, kf-outer (streams at w2 DMA arrival rate)."""
            osb = pool.tile([P, KD, n], BF16, name="osb", tag=f"osb{ci % 2}")
            ypss = []
            for kf in range(KF):
                for md in range(KD):
                    if kf == 0:
                        ypss.append(pspool.tile([P, n], F32, name="yps",
                                                tag=ps_tags[md]))
                    nc.tensor.matmul(
                        ypss[md][:], w2_at(kf)[:, md * P:(md + 1) * P],
                        hts[kf][:],
                        start=(kf == 0), stop=(kf == KF - 1),
                    )
            for md in range(KD):
                nc.vector.tensor_copy(osb[:, md, :], ypss[md][:])
                if md + 1 in batches:
                    lo = batches[md + 1]
                    nc.sync.dma_start(y_ap[:, lo:md + 1, :], osb[:, lo:md + 1, :])

        half = {4: 0, 8: 4}
        fine = {2: 0, 4: 2, 6: 4, 7: 6, 8: 7}

        wa1_at = lambda k: w1p0 if k == 0 else w13a_p[k][:, 0:F]
        wa3_at = lambda k: w3p0 if k == 0 else w13a_p[k][:, F:2 * F]

        # job 0: expert chunk 0 — fully DMA-streamed
        hts = up_kmf(wa1_at, wa3_at, lambda k: xa0_p[k], NT)
        down_kf(lambda kf: (wa2lo if kf < 2 else wa2hi)[:, kf % 2],
                hts, NT, 0, ya[0], half)
        # job 1: expert chunk 1
        hts = up_mfk(wa1_at, wa3_at, lambda k: xa1sb[:, k], NT)
        down_md(lambda kf: (wa2lo if kf < 2 else wa2hi)[:, kf % 2],
                hts, NT, 1, ya[1], half)
        # jobs 2-3: shared chunks
        hts = up_mfk(lambda k: ws13sb[:, k, 0:F], lambda k: ws13sb[:, k, F:2 * F],
                     lambda k: xs0sb[:, k], NT)
        down_md(lambda kf: ws2sb[:, kf], hts, NT, 2, ys[0], half)
        hts = up_mfk(lambda k: ws13sb[:, k, 0:F], lambda k: ws13sb[:, k, F:2 * F],
                     lambda k: xs1sb[:, k], NT)
        down_md(lambda kf: ws2sb[:, kf], hts, NT, 3, ys[1], fine)


def _get_nc():
    if "nc" not in _CACHE:
        nc = bacc.Bacc("TRN2", target_bir_lowering=False, debug=False,
                       num_devices=N_CORES)
        _emit(nc)
        nc.compile()
        _CACHE["nc"] = nc
    return _CACHE["nc"]


def _gate_numpy(x2d):
    """Replicates reference _moe_gate in float64 (routing-stable)."""
    xl = x2d.astype(np.float64)
    logits = xl @ _CACHE["gw64"].T
    scores = 1.0 / (1.0 + np.exp(-logits))
    sc = scores + _CACHE["gb64"][None, :]
    grp = sc.reshape(T, N_GROUP, E // N_GROUP)
    group_scores = np.sort(grp, axis=-1)[:, :, -2:].sum(-1)
    gidx = np.argsort(-group_scores, axis=-1, kind="stable")[:, :TOPK_GROUP]
    gmask = np.zeros((T, N_GROUP), bool)
    gmask[np.arange(T)[:, None], gidx] = True
    smask = np.repeat(gmask, E // N_GROUP, axis=1)
    tmp = np.where(smask, sc, 0.0)
    tidx = np.argsort(-tmp, axis=-1, kind="stable")[:, :TOP_K]
    tw = np.take_along_axis(scores, tidx, axis=1)
    tw = tw / (tw.sum(-1, keepdims=True) + 1e-20)
    return tidx, (tw * SCALE).astype(np.float32)


def _ffn_host(x, w1e, w2e, w3e):
    """Host fallback for overflow tokens beyond the per-core capacity."""
    h = x @ w1e.T
    h = (h / (1.0 + np.exp(-h))) * (x @ w3e.T)
    return h @ w2e.T


def _wlay_up(w):
    """[F, D] row-major -> [P, KD, F] bf16 (SBUF layout, d = k*P + p)."""
    return w.T.reshape(KD, P, F).transpose(1, 0, 2).astype(NPBF16)


def _wlay13(wg, wu):
    """gate/up [F, D] pair -> combined [P, KD, 2F] bf16."""
    return np.ascontiguousarray(
        np.concatenate([_wlay_up(wg), _wlay_up(wu)], axis=2))


def _wlay13_km(wg, wu):
    """gate/up [F, D] pair -> k-major [KD, P, 2F] bf16."""
    return np.ascontiguousarray(
        np.concatenate([_wlay_up(wg), _wlay_up(wu)], axis=2).transpose(1, 0, 2))


def _wlay_down_hm(w):
    """[D, F] row-major -> half-major [2, P, 2, D] bf16."""
    wd = _wlay_down(w)          # [P, KF, D]
    return np.ascontiguousarray(
        np.stack([wd[:, 0:2], wd[:, 2:4]]))


def _wlay_down(w):
    """[D, F] row-major -> [P, KF, D] bf16 (SBUF layout, f = k*P + p)."""
    return np.ascontiguousarray(
        w.T.reshape(KF, P, D).transpose(1, 0, 2).astype(NPBF16))


def _xlay(xrows, nchunk, clen=NT):
    """[n<=nchunk*clen, D] tokens -> [nchunk, P, KD, clen] bf16, zero-padded."""
    n = xrows.shape[0]
    full = np.zeros((nchunk * clen, D), np.float32)
    full[:n] = xrows
    return np.ascontiguousarray(
        full.reshape(nchunk, clen, KD, P).transpose(0, 3, 2, 1).astype(NPBF16))


def _ylay(y):
    """[P, KD, n] bf16 -> [n, D] fp32 (d = k*P + p)."""
    p, kd, n = y.shape
    return y.astype(np.float32).transpose(2, 1, 0).reshape(n, kd * p)


def kernel(hidden_states, gate_w, gate_bias, w1, w2, w3,
           shared_gate_w, shared_up_w, shared_down_w):
    hidden_states = np.ascontiguousarray(np.asarray(hidden_states, np.float32))
    gate_w = np.asarray(gate_w, np.float32)
    gate_bias = np.asarray(gate_bias, np.float32)
    w1 = np.asarray(w1, np.float32)
    w2 = np.asarray(w2, np.float32)
    w3 = np.asarray(w3, np.float32)
    shared_gate_w = np.asarray(shared_gate_w, np.float32)
    shared_up_w = np.asarray(shared_up_w, np.float32)
    shared_down_w = np.asarray(shared_down_w, np.float32)

    _CACHE["gw64"] = gate_w.astype(np.float64)
    _CACHE["gb64"] = gate_bias.astype(np.float64)

    x2d = hidden_states.reshape(T, D)
    tidx, tw = _gate_numpy(x2d)

    # --- per-expert token lists; overflow beyond ACAP goes to the host ---
    a_rows, a_wts, overflow_host = [], [], []
    for e in range(E):
        rows, slots = np.nonzero(tidx == e)
        wts = tw[rows, slots]
        a_rows.append(rows[:ACAP])
        a_wts.append(wts[:ACAP])
        if len(rows) > ACAP:
            overflow_host.append((e, rows[ACAP:], wts[ACAP:]))

    # --- build per-core inputs ---
    w13 = [_wlay13_km(w1[e], w3[e]) for e in range(E)]
    wdn = [_wlay_down_hm(w2[e]) for e in range(E)]
    s13 = [_wlay13(shared_gate_w[h * F:(h + 1) * F],
                   shared_up_w[h * F:(h + 1) * F]) for h in range(2)]
    s2 = [_wlay_down(shared_down_w[:, h * F:(h + 1) * F]) for h in range(2)]

    in_maps = []
    for c in range(N_CORES):
        h = c // 4
        ssl = slice((c % 4) * SSLICE, (c % 4 + 1) * SSLICE)
        xac = _xlay(x2d[a_rows[c]], 2)
        in_maps.append({
            "wa13": w13[c], "wa2": wdn[c],
            "ws13": s13[h], "ws2": s2[h],
            "xa0": np.ascontiguousarray(xac[0].transpose(1, 0, 2)),
            "xa1": xac[1],
            "xs": _xlay(x2d[ssl], 2),
        })

    nc = _get_nc()
    res = bass_utils.run_bass_kernel_spmd(
        nc, in_maps, core_ids=list(range(N_CORES))
    )
    _CACHE["last_res"] = res

    y = np.zeros((T, D), np.float32)
    for c in range(N_CORES):
        out = res.results[c]
        n = len(a_rows[c])
        yav = out["ya"]         # [2, P, KD, NT] bf16
        yaf = np.concatenate([_ylay(yav[0]), _ylay(yav[1])], axis=0)[:n]
        y[a_rows[c]] += a_wts[c][:, None] * yaf
        ssl = slice((c % 4) * SSLICE, (c % 4 + 1) * SSLICE)
        ysx = out["ys"]
        y[ssl] += np.concatenate([_ylay(ysx[0]), _ylay(ysx[1])], axis=0)
    for e, rows, wts in overflow_host:
        y[rows] += wts[:, None] * _ffn_host(x2d[rows], w1[e], w2[e], w3[e])

    return y.reshape(B, S, D)


# revision 16
# speedup vs baseline: 1.0225x; 1.0225x over previous
"""Expert-parallel MoE (Kimi/DeepSeek-V3 style sparse block) on 8 trn2 NeuronCores.

Strategy (v5 — balanced, bf16, DMA-lean):
  - Host computes the sigmoid gate + group-limited top-2 routing in numpy
    float64, then packs a balanced per-core schedule of four 512-token
    matmul chunks: [own-expert 512]x2  [shared 512]x2.
    Core c owns expert c's first 1024 tokens; the small per-expert overflow
    beyond 1024 (~2% of token-expert pairs for a balanced router) is
    evaluated on the host.  The shared FFN (SH=1024) is split into two F=512
    halves; cores 0-3 run half 0 and cores 4-7 run half 1, each over a
    distinct 1024-token slice, summed on the host.
  - All weights/activations ship as bf16 in the exact SBUF layout; matmuls
    accumulate in fp32 PSUM; outputs return bf16.
  - Overlap tuning: chunk 0's weight/x loads are split per contraction tile
    and its loops run k-outer so the PE streams at DMA arrival rate; warm-up
    matmuls on a memset tile bridge the preamble (HAM stays armed); output
    DMAs are batched 2-per-chunk (4 for the last chunk) to keep the Sync
    issue queue and the drain tail short.
"""

from contextlib import ExitStack

import numpy as np
import ml_dtypes

import concourse.bacc as bacc
import concourse.tile as tile
import concourse.mybir as mybir
from concourse import bass_utils

# --- model dims (hardcoded per problem spec) ---
B, S, D = 2, 2048, 1024
T = B * S                 # 4096 tokens
E, F = 8, 512             # routed experts / expert intermediate
SH = 1024                 # shared intermediate
TOP_K, N_GROUP, TOPK_GROUP = 2, 4, 2
SCALE = 2.5

N_CORES = 8
P = 128                   # SBUF partitions
NT = 512                  # token chunk (matmul moving free dim)
KD = D // P               # 8 contraction tiles for D
KF = F // P               # 4 contraction tiles for F
ACAP = 2 * NT             # own-expert capacity per core (2 chunks)
SSLICE = 2 * NT           # shared tokens per core (2 chunks)

F32 = mybir.dt.float32
BF16 = mybir.dt.bfloat16
NPBF16 = ml_dtypes.bfloat16

_CACHE: dict = {}


def _emit(nc):
    """Per-core program: 2 own-expert chunks + 2 shared chunks."""
    wa13 = nc.dram_tensor("wa13", [KD, P, 2 * F], BF16, kind="ExternalInput").ap()
    wa2 = nc.dram_tensor("wa2", [2, P, 2, D], BF16, kind="ExternalInput").ap()
    ws13 = nc.dram_tensor("ws13", [P, KD, 2 * F], BF16, kind="ExternalInput").ap()
    ws2 = nc.dram_tensor("ws2", [P, KF, D], BF16, kind="ExternalInput").ap()
    xa0 = nc.dram_tensor("xa0", [KD, P, NT], BF16, kind="ExternalInput").ap()
    xa1 = nc.dram_tensor("xa1", [P, KD, NT], BF16, kind="ExternalInput").ap()
    xs = nc.dram_tensor("xs", [2, P, KD, NT], BF16, kind="ExternalInput").ap()
    ya = nc.dram_tensor("ya", [2, P, KD, NT], BF16, kind="ExternalOutput").ap()
    ys = nc.dram_tensor("ys", [2, P, KD, NT], BF16, kind="ExternalOutput").ap()

    silu = mybir.ActivationFunctionType.Silu

    with tile.TileContext(nc) as tc, ExitStack() as ctx:
        pool = ctx.enter_context(tc.tile_pool(name="sb", bufs=1))
        pspool = ctx.enter_context(tc.tile_pool(name="ps", bufs=1, space="PSUM"))

        ps_tags = [f"ps{i}" for i in range(8)]

        # --- HAM warm-up: matmuls on a zero tile bridge the DMA head ---
        warm = pool.tile([P, NT], BF16, name="warm", tag="warm")
        nc.any.memset(warm[:], 0)
        wps = pspool.tile([P, NT], F32, name="wps", tag=ps_tags[0])
        for _ in range(6):
            nc.tensor.matmul(wps[:], warm[:, 0:P], warm[:], start=True, stop=True)

        # --- loads: chunk 0 split per k-tile so the PE streams immediately ---
        # (k-major DRAM layouts keep every piece transfer fully contiguous)
        w13a_p, xa0_p = [], []
        wa2lo = pool.tile([P, 2, D], BF16, name="wa2lo", tag="wa2lo")
        wa2hi = pool.tile([P, 2, D], BF16, name="wa2hi", tag="wa2hi")
        for k in range(KD):
            t = pool.tile([P, 2 * F], BF16, name=f"wa13p{k}", tag=f"wa13p{k}")
            nc.sync.dma_start(t[:], wa13[k])
            w13a_p.append(t)
            t = pool.tile([P, NT], BF16, name=f"xa0p{k}", tag=f"xa0p{k}")
            nc.sync.dma_start(t[:], xa0[k])
            xa0_p.append(t)
            if k == 5:
                nc.sync.dma_start(wa2lo[:], wa2[0])
        nc.sync.dma_start(wa2hi[:], wa2[1])
        xa1sb = pool.tile([P, KD, NT], BF16, name="xa1sb", tag="xa1")
        nc.sync.dma_start(xa1sb[:], xa1)
        ws13sb = pool.tile([P, KD, 2 * F], BF16, name="ws13sb", tag="ws13")
        nc.sync.dma_start(ws13sb[:], ws13)
        xs0sb = pool.tile([P, KD, NT], BF16, name="xs0sb", tag="xs0")
        nc.sync.dma_start(xs0sb[:], xs[0])
        ws2sb = pool.tile([P, KF, D], BF16, name="ws2sb", tag="ws2")
        nc.sync.dma_start(ws2sb[:], ws2)
        xs1sb = pool.tile([P, KD, NT], BF16, name="xs1sb", tag="xs1")
        nc.sync.dma_start(xs1sb[:], xs[1])

        def up_mfk(w1_at, w3_at, x_at, n):
            """Up-projection, mf-outer (pipelines silu/mul into the stream)."""
            hts = []
            for mf in range(KF):
                h1 = pspool.tile([P, n], F32, name="h1ps", tag=ps_tags[2 * mf])
                for k in range(KD):
                    nc.tensor.matmul(
                        h1[:], w1_at(k)[:, mf * P:(mf + 1) * P], x_at(k),
                        start=(k == 0), stop=(k == KD - 1),
                    )
                h3 = pspool.tile([P, n], F32, name="h3ps", tag=ps_tags[2 * mf + 1])
                for k in range(KD):
                    nc.tensor.matmul(
                        h3[:], w3_at(k)[:, mf * P:(mf + 1) * P], x_at(k),
                        start=(k == 0), stop=(k == KD - 1),
                    )
                a = pool.tile([P, n], F32, name="asb", tag=f"a{mf % 2}")
                nc.scalar.activation(a[:], h1[:], silu)
                ht = pool.tile([P, n], BF16, name="htsb", tag=f"ht{mf}")
                nc.vector.tensor_mul(ht[:], a[:], h3[:])
                hts.append(ht)
            return hts

        def up_kmf(w1_at, w3_at, x_at, n):
            """Up-projection, k-outer (streams at DMA arrival rate)."""
            h1s, h3s = [], []
            for k in range(KD):
                for mf in range(KF):
                    if k == 0:
                        h1s.append(pspool.tile([P, n], F32, name="h1ps",
                                               tag=ps_tags[mf]))
                    nc.tensor.matmul(
                        h1s[mf][:], w1_at(k)[:, mf * P:(mf + 1) * P], x_at(k),
                        start=(k == 0), stop=(k == KD - 1),
                    )
                for mf in range(KF):
                    if k == 0:
                        h3s.append(pspool.tile([P, n], F32, name="h3ps",
                                               tag=ps_tags[4 + mf]))
                    nc.tensor.matmul(
                        h3s[mf][:], w3_at(k)[:, mf * P:(mf + 1) * P],
                        x_at(k),
                        start=(k == 0), stop=(k == KD - 1),
                    )
            hts = []
            for mf in range(KF):
                a = pool.tile([P, n], F32, name="asb", tag=f"a{mf % 2}")
                nc.scalar.activation(a[:], h1s[mf][:], silu)
                ht = pool.tile([P, n], BF16, name="htsb", tag=f"ht{mf}")
                nc.vector.tensor_mul(ht[:], a[:], h3s[mf][:])
                hts.append(ht)
            return hts

        def down_md(w2_at, hts, n, ci, y_ap, batches):
            """Down-projection, md-outer; output DMAs batched per `batches`."""
            osb = pool.tile([P, KD, n], BF16, name="osb", tag=f"osb{ci % 2}")
            for md in range(KD):
                yps = pspool.tile([P, n], F32, name="yps", tag=ps_tags[md])
                for kf in range(KF):
                    nc.tensor.matmul(
                        yps[:], w2_at(kf)[:, md * P:(md + 1) * P], hts[kf][:],
                        start=(kf == 0), stop=(kf == KF - 1),
                    )
                nc.vector.tensor_copy(osb[:, md, :], yps[:])
                if md + 1 in batches:
                    lo = batches[md + 1]
                    nc.sync.dma_start(y_ap[:, lo:md + 1, :], osb[:, lo:md + 1, :])

        def down_kf(w2_at, hts, n, ci, y_ap, batches):
            """Down-projection, kf-outer (streams at w2 DMA arrival rate)."""
            osb = pool.tile([P, KD, n], BF16, name="osb", tag=f"osb{ci % 2}")
            ypss = []
            for kf in range(KF):
                for md in range(KD):
                    if kf == 0:
                        ypss.append(pspool.tile([P, n], F32, name="yps",
                                                tag=ps_tags[md]))
                    nc.tensor.matmul(
                        ypss[md][:], w2_at(kf)[:, md * P:(md + 1) * P],
                        hts[kf][:],
                        start=(kf == 0), stop=(kf == KF - 1),
                    )
            for md in range(KD):
                nc.vector.tensor_copy(osb[:, md, :], ypss[md][:])
                if md + 1 in batches:
                    lo = batches[md + 1]
                    nc.sync.dma_start(y_ap[:, lo:md + 1, :], osb[:, lo:md + 1, :])

        half = {4: 0, 8: 4}
        fine = {2: 0, 4: 2, 6: 4, 7: 6, 8: 7}

        wa1_at = lambda k: w13a_p[k][:, 0:F]
        wa3_at = lambda k: w13a_p[k][:, F:2 * F]

        # job 0: expert chunk 0 — fully DMA-streamed
        hts = up_kmf(wa1_at, wa3_at, lambda k: xa0_p[k], NT)
        down_kf(lambda kf: (wa2lo if kf < 2 else wa2hi)[:, kf % 2],
                hts, NT, 0, ya[0], half)
        # job 1: expert chunk 1
        hts = up_mfk(wa1_at, wa3_at, lambda k: xa1sb[:, k], NT)
        down_md(lambda kf: (wa2lo if kf < 2 else wa2hi)[:, kf % 2],
                hts, NT, 1, ya[1], half)
        # jobs 2-3: shared chunks
        hts = up_mfk(lambda k: ws13sb[:, k, 0:F], lambda k: ws13sb[:, k, F:2 * F],
                     lambda k: xs0sb[:, k], NT)
        down_md(lambda kf: ws2sb[:, kf], hts, NT, 2, ys[0], half)
        hts = up_mfk(lambda k: ws13sb[:, k, 0:F], lambda k: ws13sb[:, k, F:2 * F],
                     lambda k: xs1sb[:, k], NT)
        down_md(lambda kf: ws2sb[:, kf], hts, NT, 3, ys[1], fine)


def _get_nc():
    if "nc" not in _CACHE:
        nc = bacc.Bacc("TRN2", target_bir_lowering=False, debug=False,
                       num_devices=N_CORES)
        _emit(nc)
        nc.compile()
        _CACHE["nc"] = nc
    return _CACHE["nc"]


def _gate_numpy(x2d):
    """Replicates reference _moe_gate in float64 (routing-stable)."""
    xl = x2d.astype(np.float64)
    logits = xl @ _CACHE["gw64"].T
    scores = 1.0 / (1.0 + np.exp(-logits))
    sc = scores + _CACHE["gb64"][None, :]
    grp = sc.reshape(T, N_GROUP, E // N_GROUP)
    group_scores = np.sort(grp, axis=-1)[:, :, -2:].sum(-1)
    gidx = np.argsort(-group_scores, axis=-1, kind="stable")[:, :TOPK_GROUP]
    gmask = np.zeros((T, N_GROUP), bool)
    gmask[np.arange(T)[:, None], gidx] = True
    smask = np.repeat(gmask, E // N_GROUP, axis=1)
    tmp = np.where(smask, sc, 0.0)
    tidx = np.argsort(-tmp, axis=-1, kind="stable")[:, :TOP_K]
    tw = np.take_along_axis(scores, tidx, axis=1)
    tw = tw / (tw.sum(-1, keepdims=True) + 1e-20)
    return tidx, (tw * SCALE).astype(np.float32)


def _ffn_host(x, w1e, w2e, w3e):
    """Host fallback for overflow tokens beyond the per-core capacity."""
    h = x @ w1e.T
    h = (h / (1.0 + np.exp(-h))) * (x @ w3e.T)
    return h @ w2e.T


def _wlay_up(w):
    """[F, D] row-major -> [P, KD, F] bf16 (SBUF layout, d = k*P + p)."""
    return w.T.reshape(KD, P, F).transpose(1, 0, 2).astype(NPBF16)


def _wlay13(wg, wu):
    """gate/up [F, D] pair -> combined [P, KD, 2F] bf16."""
    return np.ascontiguousarray(
        np.concatenate([_wlay_up(wg), _wlay_up(wu)], axis=2))


def _wlay13_km(wg, wu):
    """gate/up [F, D] pair -> k-major [KD, P, 2F] bf16."""
    return np.ascontiguousarray(
        np.concatenate([_wlay_up(wg), _wlay_up(wu)], axis=2).transpose(1, 0, 2))


def _wlay_down_hm(w):
    """[D, F] row-major -> half-major [2, P, 2, D] bf16."""
    wd = _wlay_down(w)          # [P, KF, D]
    return np.ascontiguousarray(
        np.stack([wd[:, 0:2], wd[:, 2:4]]))


def _wlay_down(w):
    """[D, F] row-major -> [P, KF, D] bf16 (SBUF layout, f = k*P + p)."""
    return np.ascontiguousarray(
        w.T.reshape(KF, P, D).transpose(1, 0, 2).astype(NPBF16))


def _xlay(xrows, nchunk, clen=NT):
    """[n<=nchunk*clen, D] tokens -> [nchunk, P, KD, clen] bf16, zero-padded."""
    n = xrows.shape[0]
    full = np.zeros((nchunk * clen, D), np.float32)
    full[:n] = xrows
    return np.ascontiguousarray(
        full.reshape(nchunk, clen, KD, P).transpose(0, 3, 2, 1).astype(NPBF16))


def _ylay(y):
    """[P, KD, n] bf16 -> [n, D] fp32 (d = k*P + p)."""
    p, kd, n = y.shape
    return y.astype(np.float32).transpose(2, 1, 0).reshape(n, kd * p)


def kernel(hidden_states, gate_w, gate_bias, w1, w2, w3,
           shared_gate_w, shared_up_w, shared_down_w):
    hidden_states = np.ascontiguousarray(np.asarray(hidden_states, np.float32))
    gate_w = np.asarray(gate_w, np.float32)
    gate_bias = np.asarray(gate_bias, np.float32)
    w1 = np.asarray(w1, np.float32)
    w2 = np.asarray(w2, np.float32)
    w3 = np.asarray(w3, np.float32)
    shared_gate_w = np.asarray(shared_gate_w, np.float32)
    shared_up_w = np.asarray(shared_up_w, np.float32)
    shared_down_w = np.asarray(shared_down_w, np.float32)

    _CACHE["gw64"] = gate_w.astype(np.float64)
    _CACHE["gb64"] = gate_bias.astype(np.float64)

    x2d = hidden_states.reshape(T, D)
    tidx, tw = _gate_numpy(x2d)

    # --- per-expert token lists; overflow beyond ACAP goes to the host ---
    a_rows, a_wts, overflow_host = [], [], []
    for e in range(E):
        rows, slots = np.nonzero(tidx == e)
        wts = tw[rows, slots]
        a_rows.append(rows[:ACAP])
        a_wts.append(wts[:ACAP])
        if len(rows) > ACAP:
            overflow_host.append((e, rows[ACAP:], wts[ACAP:]))

    # --- build per-core inputs ---
    w13 = [_wlay13_km(w1[e], w3[e]) for e in range(E)]
    wdn = [_wlay_down_hm(w2[e]) for e in range(E)]
    s13 = [_wlay13(shared_gate_w[h * F:(h + 1) * F],
                   shared_up_w[h * F:(h + 1) * F]) for h in range(2)]
    s2 = [_wlay_down(shared_down_w[:, h * F:(h + 1) * F]) for h in range(2)]

    in_maps = []
    for c in range(N_CORES):
        h = c // 4
        ssl = slice((c % 4) * SSLICE, (c % 4 + 1) * SSLICE)
        xac = _xlay(x2d[a_rows[c]], 2)
        in_maps.append({
            "wa13": w13[c], "wa2": wdn[c],
            "ws13": s13[h], "ws2": s2[h],
            "xa0": np.ascontiguousarray(xac[0].transpose(1, 0, 2)),
            "xa1": xac[1],
            "xs": _xlay(x2d[ssl], 2),
        })

    nc = _get_nc()
    res = bass_utils.run_bass_kernel_spmd(
        nc, in_maps, core_ids=list(range(N_CORES))
    )
    _CACHE["last_res"] = res

    y = np.zeros((T, D), np.float32)
    for c in range(N_CORES):
        out = res.results[c]
        n = len(a_rows[c])
        yav = out["ya"]         # [2, P, KD, NT] bf16
        yaf = np.concatenate([_ylay(yav[0]), _ylay(yav[1])], axis=0)[:n]
        y[a_rows[c]] += a_wts[c][:, None] * yaf
        ssl = slice((c % 4) * SSLICE, (c % 4 + 1) * SSLICE)
        ysx = out["ys"]
        y[ssl] += np.concatenate([_ylay(ysx[0]), _ylay(ysx[1])], axis=0)
    for e, rows, wts in overflow_host:
        y[rows] += wts[:, None] * _ffn_host(x2d[rows], w1[e], w2[e], w3[e])

    return y.reshape(B, S, D)
